# revision 99
# baseline (speedup 1.0000x reference)
"""BLEU-precision loss kernel for Trainium2 (8 NeuronCores, data parallel).

loss = 1 - mean_i |set(pred_i) & set(tgt_i)| / |set(pred_i)|   over 64 rows.

Per core (8 rows): decompose each token t < 32000 into digits
lo = t & 127 (< 128) and b = t >> 7 (< 250). For each row and side, two
fp16 DVE is_equal ops build the digit one-hots in m-major tiles
oh[p, m*16 + f] = (dig[p, f] == m) against prebuilt iota tiles (dense
step-1 APs keep the DVE in 2x mode). The PE
accumulates the vocab count grid G[lo, b] = sum_k oh_lo(k)^T oh_b(k) over 16
chunks of 128 tokens per row (PSUM f32, exact). A vocab id is present iff
G > 0:
    setsize = sum(G_pred > 0),   overlap = sum((G_pred > 0) * (G_tgt > 0)).
Presence/reduce tails are interleaved per row into the one-hot/matmul
pipeline. Per-(partition,row) partial sums go to the host, which finishes
the tiny exact integer reductions and the final mean.

The kernel is DVE-bound (~91% busy): the is_equal one-hot build runs at the
fp16 2x-mode rate and nothing else on TRN2 can execute it (Pool rejects
TensorTensor at the ISA level; ACT has no equality op). The first row-side's
lo one-hot is emitted before the tgt digit extraction, and the last tile is
emitted in f-quarters so the final PE chain trails the DVE closely.
"""
import sys

sys.path.insert(0, "/opt/trn_rl_repo")

import numpy as np
import concourse.bass as bass
import concourse.bacc as bacc
from concourse import mybir

B = 64          # total rows
ROWS = 8        # rows per core
N_CORES = 8
L = 2048        # tokens per row
V = 32000       # vocab
P = 128
NB = 250        # b-digit range
W = P + NB      # combined one-hot width per token (378)
GSTRIDE = 256   # f32 row stride inside the PSUM grid (1KB, bank aligned)
CH = 16         # chunks of 128 tokens per row
F = mybir.dt.float32
F16 = mybir.dt.float16
I32 = mybir.dt.int32

_CACHE = {}


def _build_kernel():
    nc = bacc.Bacc()
    pred = nc.dram_tensor("pred", [ROWS, L], I32, kind="ExternalInput")
    tgt = nc.dram_tensor("tgt", [ROWS, L], I32, kind="ExternalInput")
    out = nc.dram_tensor("out", [P, 16], F, kind="ExternalOutput")

    # SBUF token layout per side: partition = (token index within row)//16,
    # free = row*16 + (token index)%16
    def tok_src(t):
        a = t[:]
        return bass.AP(a.tensor, 0, [[16, P], [L, ROWS], [1, 16]])

    from contextlib import ExitStack

    es = ExitStack()
    with es:
        sb = lambda name, shape, dt: es.enter_context(nc.sbuf_tensor(name, shape, dt))
        ps = lambda name, shape, dt: es.enter_context(nc.psum_tensor(name, shape, dt))
        sem = lambda name: es.enter_context(nc.semaphore(name))
        # both sides in one tile: cols 0..128 pred, 128..256 tgt, so digit
        # extraction runs as 4 wide ops instead of 8 narrow ones
        tok_a = sb("tok_a", [P, 2 * P], I32)
        lo_i = sb("lo_i", [P, 2 * P], I32)
        b_i = sb("b_i", [P, 2 * P], I32)
        # per side: cols off + 16r..16r+16 = row r's lo digits
        lo_a = sb("lo_a", [P, 2 * P], F16)
        bs_a = sb("bs_a", [P, 2 * P], F16)
        iota_c = sb("iota_c", [P, CH * W], F16)
        # each buffer holds a PAIR of row-side tiles (slot-major), so one-hot
        # is_equal ops cover two row-sides at once (half the per-op overhead)
        TW = CH * W
        oh0 = sb("oh0", [P, 2 * TW], F16)
        oh1 = sb("oh1", [P, 2 * TW], F16)
        oh2 = sb("oh2", [P, 2 * TW], F16)
        pres_p = sb("pres_p", [P, ROWS * NB], F16)
        pres_t = sb("pres_t", [P, ROWS * NB], F16)
        junk = sb("junk", [P, ROWS * NB], F16)
        res = sb("res", [P, 16], F)
        # one 2KB PSUM bank per tensor. Rows 1..6: bank r holds that row's
        # pred grid at [0,250) and tgt grid at [256,506). Rows 0 and 7 are
        # split across banks 0 and 7 (pred in bank0, tgt in bank7) so the
        # row-7 pred Sign can run while the PE still accumulates the row-7
        # tgt chain.
        G_r = [ps(f"G{r}", [P, 2 * GSTRIDE], F) for r in range(ROWS)]

        def g_slot(r, side):
            if r == 0:
                return (G_r[0], 0) if side == "p" else (G_r[7], 0)
            if r == ROWS - 1:
                return (G_r[0], GSTRIDE) if side == "p" else (G_r[7], GSTRIDE)
            return (G_r[r], 0 if side == "p" else GSTRIDE)
        s_load_p = sem("s_load_p")
        s_load_t = sem("s_load_t")
        s_pool = sem("s_pool")
        s_dve = sem("s_dve")
        s_pe = sem("s_pe")
        s_act = sem("s_act")
        s_out = sem("s_out")
        s_cast = sem("s_cast")
        block = es.enter_context(nc.Block())

        oh = (oh0, oh1, oh2)
        soff = {"p": 0, "t": P}
        # pair tiles T0..T5 cover rows 0-5: pred pair then tgt pair per row
        # pair j (rows 2j, 2j+1). Rows 6 and 7 stay single-tile so the final
        # chain can trail the DVE tightly. Chain order (s_pe counts): T_k ->
        # chains 2k+1, 2k+2; then 13:p6, 14:t6, 15:p7, 16:t7.
        PAIRS = [("p", 0), ("t", 0), ("p", 1), ("t", 1), ("p", 2), ("t", 2)]
        tickT = [0] * 6
        tickS = {}
        ticks = {"prod": [0] * ROWS}

        @block.gpsimd
        def _(g):
            # combined m-major iota: addr m*CH+f -> value m for m in [0,128)
            # (lo half) then [128,378) (bs half). fp16 exact for ints < 2048.
            g.iota(iota_c[:, :CH * P].rearrange("p (m f) -> p m f", f=CH),
                   pattern=[[1, P], [0, CH]], base=0, channel_multiplier=0,
                   allow_small_or_imprecise_dtypes=True).then_inc(s_pool, 1)
            HB = NB // 2
            g.iota(iota_c[:, CH * P:CH * (P + HB)].rearrange("p (m f) -> p m f", f=CH),
                   pattern=[[1, HB], [0, CH]], base=0, channel_multiplier=0,
                   allow_small_or_imprecise_dtypes=True).then_inc(s_pool, 1)
            g.iota(iota_c[:, CH * (P + HB):].rearrange("p (m f) -> p m f", f=CH),
                   pattern=[[1, NB - HB], [0, CH]], base=HB, channel_multiplier=0,
                   allow_small_or_imprecise_dtypes=True).then_inc(s_pool, 1)

        @block.vector
        def _(v):
            t = 0

            def inc(ins):
                nonlocal t
                t += 1
                return ins.then_inc(s_dve, 1)

            def digits(c0, c1, tick_key, cast_lo=False):
                # bitVec ops cannot cast on HW: compute in i32, cast via copy.
                # Only the pred lo cast (which gates the first emit) stays on
                # the DVE; the other three copies run on the ACT engine (idle
                # at kernel start), which incs s_cast per copy.
                cs = slice(c0, c1)
                inc(v.tensor_scalar(out=lo_i[:, cs], in0=tok_a[:, cs],
                                    scalar1=127, scalar2=None,
                                    op0=mybir.AluOpType.bitwise_and))
                inc(v.tensor_scalar(out=b_i[:, cs], in0=tok_a[:, cs],
                                    scalar1=7, scalar2=None,
                                    op0=mybir.AluOpType.logical_shift_right))
                ticks[tick_key] = t
                if cast_lo:
                    v.wait_ge(s_dve, t)
                    inc(v.tensor_copy(lo_a[:, cs], lo_i[:, cs]))
                    v.wait_ge(s_dve, t)

            def tail(r):
                nonlocal t
                # presence came from ACT (Sign); DVE only forms the product,
                # which ACT then reduces via Identity+accum. The LAST row's
                # tgt presence, product, and reduce all run on the DVE
                # itself (idle then) to skip ACT round-trips.
                sl = slice(NB * r, NB * (r + 1))
                if r == ROWS - 1:
                    # fused overlap indicator: min(G_t, pres_p) is 1 iff the
                    # vocab id is in both sets (G_t integer counts, pres_p in
                    # {0,1}) - one op instead of is_gt + mult
                    gt, ot_ = g_slot(r, "t")
                    v.wait_ge(s_pe, 2 * ROWS)  # final chain landed
                    v.wait_ge(s_act, 3 * ROWS - 3)  # pres_p7 (ACT tick 21)
                    inc(v.tensor_tensor(out=junk[:, sl],
                                        in0=gt[:, ot_:ot_ + NB],
                                        in1=pres_p[:, sl],
                                        op=mybir.AluOpType.min))
                    v.wait_ge(s_dve, t)
                    # free-dim reduction via tensor_scalar accum (4x mode)
                    # instead of the 1x reduce_sum
                    v.tensor_scalar(out=pres_t[:, sl], in0=junk[:, sl],
                                    scalar1=1.0, scalar2=0.0,
                                    op0=mybir.AluOpType.mult,
                                    op1=mybir.AluOpType.add,
                                    accum_out=res[:, r:r + 1])
                    t += 1
                    v.drain().then_inc(s_dve, 1)
                    ticks["final"] = t
                    return
                v.wait_ge(s_act, 2 if r == 0 else 3 * r + 1)
                inc(v.tensor_tensor(out=junk[:, sl], in0=pres_p[:, sl],
                                    in1=pres_t[:, sl],
                                    op=mybir.AluOpType.mult))
                ticks["prod"][r] = t

            def tail2(r0):
                # batched product for rows r0, r0+1 (halves the op overhead)
                nonlocal t
                sl = slice(NB * r0, NB * (r0 + 2))
                v.wait_ge(s_act, 3 * (r0 + 1) + 1)  # Sign_t(r0+1)
                inc(v.tensor_tensor(out=junk[:, sl], in0=pres_p[:, sl],
                                    in1=pres_t[:, sl],
                                    op=mybir.AluOpType.mult))
                ticks["prod"][r0] = ticks["prod"][r0 + 1] = t

            def emit_oh(buf, base, dig_src, mwidth, coloff, f0, f1):
                # oh[p, base + (coloff/CH + m)*CH + f] = (dig[p, f] == iota[m])
                nonlocal t
                o = oh[buf][:]
                v.tensor_tensor(
                    out=bass.AP(o.tensor, o.offset + base + coloff + f0,
                                [o.ap[0], [CH, mwidth], [1, f1 - f0]]),
                    in0=bass.AP(dig_src.tensor, dig_src.offset + f0,
                                [dig_src.ap[0], [0, mwidth], [1, f1 - f0]]),
                    in1=bass.AP(iota_c[:].tensor, coloff + f0,
                                [iota_c[:].ap[0], [CH, mwidth],
                                 [1, f1 - f0]]),
                    op=mybir.AluOpType.is_equal).then_inc(s_dve, 1)
                t += 1

            def emit_pair(buf, dig_t, dig_off, mwidth, coloff):
                # two row-side slots in one op:
                # oh[p, s*TW + coloff + m*CH + f] = (dig[p, dig_off+16s+f]==m)
                nonlocal t
                o = oh[buf][:]
                da = dig_t[:]
                v.tensor_tensor(
                    out=bass.AP(o.tensor, o.offset + coloff,
                                [o.ap[0], [TW, 2], [CH, mwidth], [1, CH]]),
                    in0=bass.AP(da.tensor, da.offset + dig_off,
                                [da.ap[0], [16, 2], [0, mwidth], [1, CH]]),
                    in1=bass.AP(iota_c[:].tensor, coloff,
                                [iota_c[:].ap[0], [0, 2], [CH, mwidth],
                                 [1, CH]]),
                    op=mybir.AluOpType.is_equal).then_inc(s_dve, 1)
                t += 1

            HB = NB // 2
            v.wait_ge(s_load_p, 16)
            digits(0, P, "dig_p", cast_lo=True)
            # T0's lo pair only needs pred lo digits + the lo iota; emit it
            # before the tgt digit extraction to start the pipeline earlier
            v.wait_ge(s_pool, 1)  # iota lo half ready
            emit_pair(0, lo_a, 0, P, 0)
            v.wait_ge(s_load_t, 16)
            digits(P, 2 * P, "dig_t")
            # T0's b pair in m-halves; compare each as its iota lands
            v.wait_ge(s_cast, 1)  # pred b cast (on ACT) done
            v.wait_ge(s_pool, 2)
            emit_pair(0, bs_a, 0, HB, CH * P)
            v.wait_ge(s_pool, 3)
            emit_pair(0, bs_a, 0, NB - HB, CH * (P + HB))
            tickT[0] = t

            for k in range(1, 6):
                side, j = PAIRS[k]
                if k == 1:
                    v.wait_ge(s_cast, 3)  # tgt digit casts (on ACT) done
                if k >= 3:
                    v.wait_ge(s_pe, 2 * (k - 3) + 2)  # buffer k%3 consumed
                off = soff[side] + 32 * j
                emit_pair(k % 3, lo_a, off, P, 0)
                emit_pair(k % 3, bs_a, off, NB, CH * P)
                tickT[k] = t
                if k == 3:
                    tail2(0)
                elif k == 4:
                    tail2(2)

            # singles: S12=p6, S13=t6 reuse buf0 (T3, free after chain 8);
            # S14=p7, S15=t7 reuse buf1 (T4, free after chain 10)
            def dig16(side, r):
                o = soff[side] + 16 * r
                return lo_a[:, o:o + 16], bs_a[:, o:o + 16]

            v.wait_ge(s_pe, 8)
            lo_s, bs_s = dig16("p", 6)
            emit_oh(0, 0, lo_s, P, 0, 0, CH)
            emit_oh(0, 0, bs_s, NB, CH * P, 0, CH)
            tickS[12] = t
            lo_s, bs_s = dig16("t", 6)
            emit_oh(0, TW, lo_s, P, 0, 0, CH)
            emit_oh(0, TW, bs_s, NB, CH * P, 0, CH)
            tickS[13] = t
            tail2(4)
            v.wait_ge(s_pe, 10)
            lo_s, bs_s = dig16("p", 7)
            emit_oh(1, 0, lo_s, P, 0, 0, CH)
            emit_oh(1, 0, bs_s, NB, CH * P, 0, CH)
            tickS[14] = t
            # last row-side in f-quarters so the final PE chain trails the
            # DVE closely instead of waiting for the full tile; row 6's
            # product goes AFTER the emits, into the wait-for-final-chain gap
            lo_s, bs_s = dig16("t", 7)
            ticks["oh_q"] = {}
            for f0, f1 in ((0, 4), (4, 8), (8, 12), (12, 14), (14, 16)):
                emit_oh(1, TW, lo_s, P, 0, f0, f1)
                emit_oh(1, TW, bs_s, NB, CH * P, f0, f1)
                ticks["oh_q"][f1] = t
            tail(6)

            tail(ROWS - 1)

        @block.scalar
        def _(sc):

            # tgt digit casts offloaded from the DVE (ACT is idle here);
            # signalled via s_pool so the s_act Sign/Id tick numbering is
            # untouched
            sc.wait_ge(s_dve, ticks["dig_p"])
            sc.activation(bs_a[:, :P], b_i[:, :P],
                          mybir.ActivationFunctionType.Copy,
                          ).then_inc(s_cast, 1)
            sc.wait_ge(s_dve, ticks["dig_t"])
            sc.activation(lo_a[:, P:], lo_i[:, P:],
                          mybir.ActivationFunctionType.Copy,
                          ).then_inc(s_cast, 1)
            sc.activation(bs_a[:, P:], b_i[:, P:],
                          mybir.ActivationFunctionType.Copy,
                          ).then_inc(s_cast, 1)
            # chain numbering under the pair schedule: rows r<6 have
            # c_p = 4(r//2)+1+r%2, c_t = c_p+2; then 13:p6, 14:t6, 15:p7,
            # 16:t7. A bank may be read only when ALL its writer chains are
            # done (rows 1..6 share a bank between their p and t grids; rows
            # 0 and 7 use the split banks, readable after their own chain).
            W_p = {0: 1, 1: 4, 2: 7, 3: 8, 4: 11, 5: 12, 6: 14, 7: 15}
            W_t = {0: 3, 1: 4, 2: 7, 3: 8, 4: 11, 5: 12, 6: 14}
            for r in range(ROWS):
                sl = slice(NB * r, NB * (r + 1))
                gp, op_ = g_slot(r, "p")
                gt, ot_ = g_slot(r, "t")
                sc.wait_ge(s_pe, W_p[r])
                sc.activation(pres_p[:, sl], gp[:, op_:op_ + NB],
                              mybir.ActivationFunctionType.Sign,
                              accum_out=res[:, ROWS + r:ROWS + r + 1],
                              ).then_inc(s_act, 1)
                if r < ROWS - 1:
                    sc.wait_ge(s_pe, W_t[r])
                    sc.activation(pres_t[:, sl], gt[:, ot_:ot_ + NB],
                                  mybir.ActivationFunctionType.Sign,
                                  ).then_inc(s_act, 1)
                if r >= 1:
                    sc.wait_ge(s_dve, ticks["prod"][r - 1])
                    sc.activation(junk[:, NB * (r - 1):NB * r],
                                  junk[:, NB * (r - 1):NB * r],
                                  mybir.ActivationFunctionType.Identity,
                                  accum_out=res[:, r - 1:r],
                                  ).then_inc(s_act, 1)

        @block.tensor
        def _(te):
            def chain(bufi, base, gten, goff, last=False):
                a = oh[bufi][:]
                nxt = {4: 8, 8: 12, 12: 14, 14: 16}
                for f in range(CH):
                    if last and f in nxt:
                        te.wait_ge(s_dve, ticks["oh_q"][nxt[f]])
                    ins = te.matmul(
                        out=gten[:, goff:goff + NB],
                        lhsT=bass.AP(a.tensor, a.offset + base + f,
                                     [a.ap[0], [CH, P]]),
                        rhs=bass.AP(a.tensor, a.offset + base + CH * P + f,
                                    [a.ap[0], [CH, NB]]),
                        start=(f == 0),
                        stop=(f == CH - 1),
                    )
                    if f == CH - 1:
                        ins.then_inc(s_pe, 1)

            for k in range(6):
                side, j = PAIRS[k]
                te.wait_ge(s_dve, tickT[k])
                for s in range(2):
                    g, go = g_slot(2 * j + s, side)
                    chain(k % 3, s * TW, g, go)
            te.wait_ge(s_dve, tickS[12])
            g, go = g_slot(6, "p")
            chain(0, 0, g, go)
            te.wait_ge(s_dve, tickS[13])
            g, go = g_slot(6, "t")
            chain(0, TW, g, go)
            te.wait_ge(s_dve, tickS[14])
            g, go = g_slot(7, "p")
            chain(1, 0, g, go)
            te.wait_ge(s_dve, ticks["oh_q"][4])
            g, go = g_slot(7, "t")
            chain(1, TW, g, go, last=True)

        @block.sync
        def _(sy):
            sy.dma_start(out=tok_a[:, :P].rearrange("p (a g) -> p a g", g=16),
                         in_=tok_src(pred)).then_inc(s_load_p, 16)
            # tgt load issued second on the same engine: pred's DMA wins the
            # shared HWDGE deterministically (tgt digits aren't read until
            # after the first lo emit, ~5us in)
            sy.dma_start(out=tok_a[:, P:].rearrange("p (a g) -> p a g", g=16),
                         in_=tok_src(tgt)).then_inc(s_load_t, 16)
            # rows 0..5 fully reduced once red5 fired (ACT tick 3*6+2=20)
            sy.wait_ge(s_act, 20)
            ra = res[:]
            sy.dma_start(
                out=bass.AP(out[:].tensor, 0, [[16, P], [ROWS, 2], [1, 6]]),
                in_=bass.AP(ra.tensor, 0, [ra.ap[0], [ROWS, 2], [1, 6]]),
            ).then_inc(s_out, 16)
            sy.wait_ge(s_act, 3 * ROWS - 2)
            sy.wait_ge(s_dve, ticks["final"])
            sy.dma_start(
                out=bass.AP(out[:].tensor, 6, [[16, P], [ROWS, 2], [1, 2]]),
                in_=bass.AP(ra.tensor, 6, [ra.ap[0], [ROWS, 2], [1, 2]]),
            ).then_inc(s_out, 16)
            sy.wait_ge(s_out, 32)

    nc.compile()
    return nc


def run(pred_tokens, tgt_tokens, trace=False):
    """Returns (loss, exec_time_ns_or_None)."""
    from concourse.bass_utils import run_bass_kernel_spmd

    if "nc" not in _CACHE:
        _CACHE["nc"] = _build_kernel()
    nc = _CACHE["nc"]

    pred_tokens = np.ascontiguousarray(np.asarray(pred_tokens, dtype=np.int32))
    tgt_tokens = np.ascontiguousarray(np.asarray(tgt_tokens, dtype=np.int32))
    assert pred_tokens.shape == (B, L) and tgt_tokens.shape == (B, L)

    in_maps = [
        {
            "pred": pred_tokens[c * ROWS:(c + 1) * ROWS],
            "tgt": tgt_tokens[c * ROWS:(c + 1) * ROWS],
        }
        for c in range(N_CORES)
    ]
    try:
        kres = run_bass_kernel_spmd(nc, in_maps, list(range(N_CORES)),
                                    trace=trace)
    except ModuleNotFoundError:
        # NTFF profiling hook unavailable in this axon client
        kres = run_bass_kernel_spmd(nc, in_maps, list(range(N_CORES)))

    ov = np.empty(B, dtype=np.float32)
    ss = np.empty(B, dtype=np.float32)
    for c, r in enumerate(kres.results):
        o = r["out"]  # [128, 16] f32: cols 0..7 overlap partials, 8..15 setsize
        ov[c * ROWS:(c + 1) * ROWS] = o[:, :ROWS].sum(axis=0, dtype=np.float64)
        ss[c * ROWS:(c + 1) * ROWS] = o[:, ROWS:].sum(axis=0, dtype=np.float64)

    precision = np.where(ss > 0, ov / np.maximum(ss, np.float32(1.0)),
                         np.float32(0.0)).astype(np.float32)
    loss = np.float32(1.0) - np.float32(precision.mean(dtype=np.float64))
    return loss, kres.exec_time_ns


def kernel(pred_tokens, target_tokens):
    loss, _ = run(pred_tokens, target_tokens)
    return loss


if __name__ == "__main__":
    rng = np.random.default_rng(0)
    p = rng.integers(0, V, (B, L), dtype=np.int32)
    t = rng.integers(0, V, (B, L), dtype=np.int32)
    print(kernel(p, t))



# revision 101
# speedup vs baseline: 1.0020x; 1.0020x over previous
"""BLEU-precision loss kernel for Trainium2 (8 NeuronCores, data parallel).

loss = 1 - mean_i |set(pred_i) & set(tgt_i)| / |set(pred_i)|   over 64 rows.

Per core (8 rows): decompose each token t < 32000 into digits
lo = t & 127 (< 128) and b = t >> 7 (< 250). For each row and side, two
fp16 DVE is_equal ops build the digit one-hots in m-major tiles
oh[p, m*16 + f] = (dig[p, f] == m) against prebuilt iota tiles (dense
step-1 APs keep the DVE in 2x mode). The PE
accumulates the vocab count grid G[lo, b] = sum_k oh_lo(k)^T oh_b(k) over 16
chunks of 128 tokens per row (PSUM f32, exact). A vocab id is present iff
G > 0:
    setsize = sum(G_pred > 0),   overlap = sum((G_pred > 0) * (G_tgt > 0)).
Presence/reduce tails are interleaved per row into the one-hot/matmul
pipeline. Per-(partition,row) partial sums go to the host, which finishes
the tiny exact integer reductions and the final mean.

The kernel is DVE-bound (~91% busy): the is_equal one-hot build runs at the
fp16 2x-mode rate and nothing else on TRN2 can execute it (Pool rejects
TensorTensor at the ISA level; ACT has no equality op). The first row-side's
lo one-hot is emitted before the tgt digit extraction, and the last tile is
emitted in f-quarters so the final PE chain trails the DVE closely.
"""
import sys

sys.path.insert(0, "/opt/trn_rl_repo")

import numpy as np
import concourse.bass as bass
import concourse.bacc as bacc
from concourse import mybir

B = 64          # total rows
ROWS = 8        # rows per core
N_CORES = 8
L = 2048        # tokens per row
V = 32000       # vocab
P = 128
NB = 250        # b-digit range
W = P + NB      # combined one-hot width per token (378)
GSTRIDE = 256   # f32 row stride inside the PSUM grid (1KB, bank aligned)
CH = 16         # chunks of 128 tokens per row
F = mybir.dt.float32
F16 = mybir.dt.float16
I32 = mybir.dt.int32

_CACHE = {}


def _build_kernel():
    nc = bacc.Bacc()
    pred = nc.dram_tensor("pred", [ROWS, L], I32, kind="ExternalInput")
    tgt = nc.dram_tensor("tgt", [ROWS, L], I32, kind="ExternalInput")
    out = nc.dram_tensor("out", [P, 16], F, kind="ExternalOutput")

    # SBUF token layout per side: partition = (token index within row)//16,
    # free = row*16 + (token index)%16
    def tok_src(t):
        a = t[:]
        return bass.AP(a.tensor, 0, [[16, P], [L, ROWS], [1, 16]])

    from contextlib import ExitStack

    es = ExitStack()
    with es:
        sb = lambda name, shape, dt: es.enter_context(nc.sbuf_tensor(name, shape, dt))
        ps = lambda name, shape, dt: es.enter_context(nc.psum_tensor(name, shape, dt))
        sem = lambda name: es.enter_context(nc.semaphore(name))
        # both sides in one tile: cols 0..128 pred, 128..256 tgt, so digit
        # extraction runs as 4 wide ops instead of 8 narrow ones
        tok_a = sb("tok_a", [P, 2 * P], I32)
        lo_i = sb("lo_i", [P, 2 * P], I32)
        b_i = sb("b_i", [P, 2 * P], I32)
        # per side: cols off + 16r..16r+16 = row r's lo digits
        lo_a = sb("lo_a", [P, 2 * P], F16)
        bs_a = sb("bs_a", [P, 2 * P], F16)
        iota_c = sb("iota_c", [P, CH * W], F16)
        # each buffer holds a PAIR of row-side tiles (slot-major), so one-hot
        # is_equal ops cover two row-sides at once (half the per-op overhead)
        TW = CH * W
        oh0 = sb("oh0", [P, 2 * TW], F16)
        oh1 = sb("oh1", [P, 2 * TW], F16)
        oh2 = sb("oh2", [P, 2 * TW], F16)
        pres_p = sb("pres_p", [P, ROWS * NB], F16)
        pres_t = sb("pres_t", [P, ROWS * NB], F16)
        junk = sb("junk", [P, ROWS * NB], F16)
        res = sb("res", [P, 16], F)
        # one 2KB PSUM bank per tensor. Rows 1..6: bank r holds that row's
        # pred grid at [0,250) and tgt grid at [256,506). Rows 0 and 7 are
        # split across banks 0 and 7 (pred in bank0, tgt in bank7) so the
        # row-7 pred Sign can run while the PE still accumulates the row-7
        # tgt chain.
        G_r = [ps(f"G{r}", [P, 2 * GSTRIDE], F) for r in range(ROWS)]

        def g_slot(r, side):
            if r == 0:
                return (G_r[0], 0) if side == "p" else (G_r[7], 0)
            if r == ROWS - 1:
                return (G_r[0], GSTRIDE) if side == "p" else (G_r[7], GSTRIDE)
            return (G_r[r], 0 if side == "p" else GSTRIDE)
        s_load_p = sem("s_load_p")
        s_load_t = sem("s_load_t")
        s_pool = sem("s_pool")
        s_dve = sem("s_dve")
        s_pe = sem("s_pe")
        s_act = sem("s_act")
        s_out = sem("s_out")
        s_cast = sem("s_cast")
        block = es.enter_context(nc.Block())

        oh = (oh0, oh1, oh2)
        soff = {"p": 0, "t": P}
        # pair tiles T0..T5 cover rows 0-5: pred pair then tgt pair per row
        # pair j (rows 2j, 2j+1). Rows 6 and 7 stay single-tile so the final
        # chain can trail the DVE tightly. Chain order (s_pe counts): T_k ->
        # chains 2k+1, 2k+2; then 13:p6, 14:t6, 15:p7, 16:t7.
        PAIRS = [("p", 0), ("t", 0), ("p", 1), ("t", 1), ("p", 2), ("t", 2)]
        tickT = [0] * 6
        tickS = {}
        ticks = {"prod": [0] * ROWS}

        @block.gpsimd
        def _(g):
            # combined m-major iota: addr m*CH+f -> value m for m in [0,128)
            # (lo half) then [128,378) (bs half). fp16 exact for ints < 2048.
            g.iota(iota_c[:, :CH * P].rearrange("p (m f) -> p m f", f=CH),
                   pattern=[[1, P], [0, CH]], base=0, channel_multiplier=0,
                   allow_small_or_imprecise_dtypes=True).then_inc(s_pool, 1)
            HB = NB // 2
            g.iota(iota_c[:, CH * P:CH * (P + HB)].rearrange("p (m f) -> p m f", f=CH),
                   pattern=[[1, HB], [0, CH]], base=0, channel_multiplier=0,
                   allow_small_or_imprecise_dtypes=True).then_inc(s_pool, 1)
            g.iota(iota_c[:, CH * (P + HB):].rearrange("p (m f) -> p m f", f=CH),
                   pattern=[[1, NB - HB], [0, CH]], base=HB, channel_multiplier=0,
                   allow_small_or_imprecise_dtypes=True).then_inc(s_pool, 1)

        @block.vector
        def _(v):
            t = 0

            def inc(ins):
                nonlocal t
                t += 1
                return ins.then_inc(s_dve, 1)

            def digits(c0, c1, tick_key, cast_lo=False):
                # bitVec ops cannot cast on HW: compute in i32, cast via copy.
                # Only the pred lo cast (which gates the first emit) stays on
                # the DVE; the other three copies run on the ACT engine (idle
                # at kernel start), which incs s_cast per copy.
                cs = slice(c0, c1)
                inc(v.tensor_scalar(out=lo_i[:, cs], in0=tok_a[:, cs],
                                    scalar1=127, scalar2=None,
                                    op0=mybir.AluOpType.bitwise_and))
                inc(v.tensor_scalar(out=b_i[:, cs], in0=tok_a[:, cs],
                                    scalar1=7, scalar2=None,
                                    op0=mybir.AluOpType.logical_shift_right))
                ticks[tick_key] = t
                if cast_lo:
                    v.wait_ge(s_dve, t)
                    inc(v.tensor_copy(lo_a[:, cs], lo_i[:, cs]))
                    v.wait_ge(s_dve, t)

            def tail(r):
                nonlocal t
                # presence came from ACT (Sign); DVE only forms the product,
                # which ACT then reduces via Identity+accum. The LAST row's
                # tgt presence, product, and reduce all run on the DVE
                # itself (idle then) to skip ACT round-trips.
                sl = slice(NB * r, NB * (r + 1))
                if r == ROWS - 1:
                    # fused overlap indicator: min(G_t, pres_p) is 1 iff the
                    # vocab id is in both sets (G_t integer counts, pres_p in
                    # {0,1}) - one op instead of is_gt + mult
                    gt, ot_ = g_slot(r, "t")
                    v.wait_ge(s_pe, 2 * ROWS)  # final chain landed
                    v.wait_ge(s_act, 3 * ROWS - 3)  # pres_p7 (ACT tick 21)
                    inc(v.tensor_tensor(out=junk[:, sl],
                                        in0=gt[:, ot_:ot_ + NB],
                                        in1=pres_p[:, sl],
                                        op=mybir.AluOpType.min))
                    v.wait_ge(s_dve, t)
                    # free-dim reduction via tensor_scalar accum (4x mode)
                    # instead of the 1x reduce_sum
                    v.tensor_scalar(out=pres_t[:, sl], in0=junk[:, sl],
                                    scalar1=1.0, scalar2=0.0,
                                    op0=mybir.AluOpType.mult,
                                    op1=mybir.AluOpType.add,
                                    accum_out=res[:, r:r + 1])
                    t += 1
                    v.drain().then_inc(s_dve, 1)
                    ticks["final"] = t
                    return
                v.wait_ge(s_act, 2 if r == 0 else 3 * r + 1)
                inc(v.tensor_tensor(out=junk[:, sl], in0=pres_p[:, sl],
                                    in1=pres_t[:, sl],
                                    op=mybir.AluOpType.mult))
                ticks["prod"][r] = t

            def tail2(r0):
                # batched product for rows r0, r0+1 (halves the op overhead)
                nonlocal t
                sl = slice(NB * r0, NB * (r0 + 2))
                v.wait_ge(s_act, 3 * (r0 + 1) + 1)  # Sign_t(r0+1)
                inc(v.tensor_tensor(out=junk[:, sl], in0=pres_p[:, sl],
                                    in1=pres_t[:, sl],
                                    op=mybir.AluOpType.mult))
                ticks["prod"][r0] = ticks["prod"][r0 + 1] = t

            def emit_oh(buf, base, dig_src, mwidth, coloff, f0, f1):
                # oh[p, base + (coloff/CH + m)*CH + f] = (dig[p, f] == iota[m])
                nonlocal t
                o = oh[buf][:]
                v.tensor_tensor(
                    out=bass.AP(o.tensor, o.offset + base + coloff + f0,
                                [o.ap[0], [CH, mwidth], [1, f1 - f0]]),
                    in0=bass.AP(dig_src.tensor, dig_src.offset + f0,
                                [dig_src.ap[0], [0, mwidth], [1, f1 - f0]]),
                    in1=bass.AP(iota_c[:].tensor, coloff + f0,
                                [iota_c[:].ap[0], [CH, mwidth],
                                 [1, f1 - f0]]),
                    op=mybir.AluOpType.is_equal).then_inc(s_dve, 1)
                t += 1

            def emit_pair(buf, dig_t, dig_off, mwidth, coloff, sstride=16):
                # two row-side slots in one op:
                # oh[p, s*TW + coloff + m*CH + f]
                #     = (dig[p, dig_off + sstride*s + f] == m)
                nonlocal t
                o = oh[buf][:]
                da = dig_t[:]
                v.tensor_tensor(
                    out=bass.AP(o.tensor, o.offset + coloff,
                                [o.ap[0], [TW, 2], [CH, mwidth], [1, CH]]),
                    in0=bass.AP(da.tensor, da.offset + dig_off,
                                [da.ap[0], [sstride, 2], [0, mwidth], [1, CH]]),
                    in1=bass.AP(iota_c[:].tensor, coloff,
                                [iota_c[:].ap[0], [0, 2], [CH, mwidth],
                                 [1, CH]]),
                    op=mybir.AluOpType.is_equal).then_inc(s_dve, 1)
                t += 1

            HB = NB // 2
            v.wait_ge(s_load_p, 16)
            digits(0, P, "dig_p", cast_lo=True)
            # T0's lo pair only needs pred lo digits + the lo iota; emit it
            # before the tgt digit extraction to start the pipeline earlier
            v.wait_ge(s_pool, 1)  # iota lo half ready
            emit_pair(0, lo_a, 0, P, 0)
            v.wait_ge(s_load_t, 16)
            digits(P, 2 * P, "dig_t")
            # T0's b pair in m-halves; compare each as its iota lands
            v.wait_ge(s_cast, 1)  # pred b cast (on ACT) done
            v.wait_ge(s_pool, 2)
            emit_pair(0, bs_a, 0, HB, CH * P)
            v.wait_ge(s_pool, 3)
            emit_pair(0, bs_a, 0, NB - HB, CH * (P + HB))
            tickT[0] = t

            for k in range(1, 6):
                side, j = PAIRS[k]
                if k == 1:
                    v.wait_ge(s_cast, 3)  # tgt digit casts (on ACT) done
                if k >= 3:
                    v.wait_ge(s_pe, 2 * (k - 3) + 2)  # buffer k%3 consumed
                off = soff[side] + 32 * j
                emit_pair(k % 3, lo_a, off, P, 0)
                emit_pair(k % 3, bs_a, off, NB, CH * P)
                tickT[k] = t
                if k == 3:
                    tail2(0)
                elif k == 4:
                    tail2(2)

            # singles: S12=p6, S13=t6 reuse buf0 (T3, free after chain 8);
            # S14=p7, S15=t7 reuse buf1 (T4, free after chain 10)
            def dig16(side, r):
                o = soff[side] + 16 * r
                return lo_a[:, o:o + 16], bs_a[:, o:o + 16]

            v.wait_ge(s_pe, 8)
            # p6 and t6 batched as one pair (digit cols 96 and 224: uniform
            # stride 128); chains 13/14 start later but chain 15 is gated by
            # the S14 tick anyway, so the end structure is unchanged
            emit_pair(0, lo_a, soff["p"] + 96, P, 0, sstride=P)
            emit_pair(0, bs_a, soff["p"] + 96, NB, CH * P, sstride=P)
            tickS[12] = tickS[13] = t
            tail2(4)
            v.wait_ge(s_pe, 10)
            lo_s, bs_s = dig16("p", 7)
            emit_oh(1, 0, lo_s, P, 0, 0, CH)
            emit_oh(1, 0, bs_s, NB, CH * P, 0, CH)
            tickS[14] = t
            # last row-side in f-quarters so the final PE chain trails the
            # DVE closely instead of waiting for the full tile; row 6's
            # product goes AFTER the emits, into the wait-for-final-chain gap
            lo_s, bs_s = dig16("t", 7)
            ticks["oh_q"] = {}
            for f0, f1 in ((0, 4), (4, 8), (8, 12), (12, 14), (14, 16)):
                emit_oh(1, TW, lo_s, P, 0, f0, f1)
                emit_oh(1, TW, bs_s, NB, CH * P, f0, f1)
                ticks["oh_q"][f1] = t
            tail(6)

            tail(ROWS - 1)

        @block.scalar
        def _(sc):

            # tgt digit casts offloaded from the DVE (ACT is idle here);
            # signalled via s_pool so the s_act Sign/Id tick numbering is
            # untouched
            sc.wait_ge(s_dve, ticks["dig_p"])
            sc.activation(bs_a[:, :P], b_i[:, :P],
                          mybir.ActivationFunctionType.Copy,
                          ).then_inc(s_cast, 1)
            sc.wait_ge(s_dve, ticks["dig_t"])
            sc.activation(lo_a[:, P:], lo_i[:, P:],
                          mybir.ActivationFunctionType.Copy,
                          ).then_inc(s_cast, 1)
            sc.activation(bs_a[:, P:], b_i[:, P:],
                          mybir.ActivationFunctionType.Copy,
                          ).then_inc(s_cast, 1)
            # chain numbering under the pair schedule: rows r<6 have
            # c_p = 4(r//2)+1+r%2, c_t = c_p+2; then 13:p6, 14:t6, 15:p7,
            # 16:t7. A bank may be read only when ALL its writer chains are
            # done (rows 1..6 share a bank between their p and t grids; rows
            # 0 and 7 use the split banks, readable after their own chain).
            W_p = {0: 1, 1: 4, 2: 7, 3: 8, 4: 11, 5: 12, 6: 14, 7: 15}
            W_t = {0: 3, 1: 4, 2: 7, 3: 8, 4: 11, 5: 12, 6: 14}
            for r in range(ROWS):
                sl = slice(NB * r, NB * (r + 1))
                gp, op_ = g_slot(r, "p")
                gt, ot_ = g_slot(r, "t")
                sc.wait_ge(s_pe, W_p[r])
                sc.activation(pres_p[:, sl], gp[:, op_:op_ + NB],
                              mybir.ActivationFunctionType.Sign,
                              accum_out=res[:, ROWS + r:ROWS + r + 1],
                              ).then_inc(s_act, 1)
                if r < ROWS - 1:
                    sc.wait_ge(s_pe, W_t[r])
                    sc.activation(pres_t[:, sl], gt[:, ot_:ot_ + NB],
                                  mybir.ActivationFunctionType.Sign,
                                  ).then_inc(s_act, 1)
                if r >= 1:
                    sc.wait_ge(s_dve, ticks["prod"][r - 1])
                    sc.activation(junk[:, NB * (r - 1):NB * r],
                                  junk[:, NB * (r - 1):NB * r],
                                  mybir.ActivationFunctionType.Identity,
                                  accum_out=res[:, r - 1:r],
                                  ).then_inc(s_act, 1)

        @block.tensor
        def _(te):
            def chain(bufi, base, gten, goff, last=False):
                a = oh[bufi][:]
                nxt = {4: 8, 8: 12, 12: 14, 14: 16}
                for f in range(CH):
                    if last and f in nxt:
                        te.wait_ge(s_dve, ticks["oh_q"][nxt[f]])
                    ins = te.matmul(
                        out=gten[:, goff:goff + NB],
                        lhsT=bass.AP(a.tensor, a.offset + base + f,
                                     [a.ap[0], [CH, P]]),
                        rhs=bass.AP(a.tensor, a.offset + base + CH * P + f,
                                    [a.ap[0], [CH, NB]]),
                        start=(f == 0),
                        stop=(f == CH - 1),
                    )
                    if f == CH - 1:
                        ins.then_inc(s_pe, 1)

            for k in range(6):
                side, j = PAIRS[k]
                te.wait_ge(s_dve, tickT[k])
                for s in range(2):
                    g, go = g_slot(2 * j + s, side)
                    chain(k % 3, s * TW, g, go)
            te.wait_ge(s_dve, tickS[12])
            g, go = g_slot(6, "p")
            chain(0, 0, g, go)
            te.wait_ge(s_dve, tickS[13])
            g, go = g_slot(6, "t")
            chain(0, TW, g, go)
            te.wait_ge(s_dve, tickS[14])
            g, go = g_slot(7, "p")
            chain(1, 0, g, go)
            te.wait_ge(s_dve, ticks["oh_q"][4])
            g, go = g_slot(7, "t")
            chain(1, TW, g, go, last=True)

        @block.sync
        def _(sy):
            sy.dma_start(out=tok_a[:, :P].rearrange("p (a g) -> p a g", g=16),
                         in_=tok_src(pred)).then_inc(s_load_p, 16)
            # tgt load issued second on the same engine: pred's DMA wins the
            # shared HWDGE deterministically (tgt digits aren't read until
            # after the first lo emit, ~5us in)
            sy.dma_start(out=tok_a[:, P:].rearrange("p (a g) -> p a g", g=16),
                         in_=tok_src(tgt)).then_inc(s_load_t, 16)
            # rows 0..5 fully reduced once red5 fired (ACT tick 3*6+2=20)
            sy.wait_ge(s_act, 20)
            ra = res[:]
            sy.dma_start(
                out=bass.AP(out[:].tensor, 0, [[16, P], [ROWS, 2], [1, 6]]),
                in_=bass.AP(ra.tensor, 0, [ra.ap[0], [ROWS, 2], [1, 6]]),
            ).then_inc(s_out, 16)
            sy.wait_ge(s_act, 3 * ROWS - 2)
            sy.wait_ge(s_dve, ticks["final"])
            sy.dma_start(
                out=bass.AP(out[:].tensor, 6, [[16, P], [ROWS, 2], [1, 2]]),
                in_=bass.AP(ra.tensor, 6, [ra.ap[0], [ROWS, 2], [1, 2]]),
            ).then_inc(s_out, 16)
            sy.wait_ge(s_out, 32)

    nc.compile()
    return nc


def run(pred_tokens, tgt_tokens, trace=False):
    """Returns (loss, exec_time_ns_or_None)."""
    from concourse.bass_utils import run_bass_kernel_spmd

    if "nc" not in _CACHE:
        _CACHE["nc"] = _build_kernel()
    nc = _CACHE["nc"]

    pred_tokens = np.ascontiguousarray(np.asarray(pred_tokens, dtype=np.int32))
    tgt_tokens = np.ascontiguousarray(np.asarray(tgt_tokens, dtype=np.int32))
    assert pred_tokens.shape == (B, L) and tgt_tokens.shape == (B, L)

    in_maps = [
        {
            "pred": pred_tokens[c * ROWS:(c + 1) * ROWS],
            "tgt": tgt_tokens[c * ROWS:(c + 1) * ROWS],
        }
        for c in range(N_CORES)
    ]
    try:
        kres = run_bass_kernel_spmd(nc, in_maps, list(range(N_CORES)),
                                    trace=trace)
    except ModuleNotFoundError:
        # NTFF profiling hook unavailable in this axon client
        kres = run_bass_kernel_spmd(nc, in_maps, list(range(N_CORES)))

    ov = np.empty(B, dtype=np.float32)
    ss = np.empty(B, dtype=np.float32)
    for c, r in enumerate(kres.results):
        o = r["out"]  # [128, 16] f32: cols 0..7 overlap partials, 8..15 setsize
        ov[c * ROWS:(c + 1) * ROWS] = o[:, :ROWS].sum(axis=0, dtype=np.float64)
        ss[c * ROWS:(c + 1) * ROWS] = o[:, ROWS:].sum(axis=0, dtype=np.float64)

    precision = np.where(ss > 0, ov / np.maximum(ss, np.float32(1.0)),
                         np.float32(0.0)).astype(np.float32)
    loss = np.float32(1.0) - np.float32(precision.mean(dtype=np.float64))
    return loss, kres.exec_time_ns


def kernel(pred_tokens, target_tokens):
    loss, _ = run(pred_tokens, target_tokens)
    return loss


if __name__ == "__main__":
    rng = np.random.default_rng(0)
    p = rng.integers(0, V, (B, L), dtype=np.int32)
    t = rng.integers(0, V, (B, L), dtype=np.int32)
    print(kernel(p, t))



# revision 104
# speedup vs baseline: 1.0041x; 1.0020x over previous
"""BLEU-precision loss kernel for Trainium2 (8 NeuronCores, data parallel).

loss = 1 - mean_i |set(pred_i) & set(tgt_i)| / |set(pred_i)|   over 64 rows.

Per core (8 rows): decompose each token t < 32000 into digits
lo = t & 127 (< 128) and b = t >> 7 (< 250). For each row and side, two
fp16 DVE is_equal ops build the digit one-hots in m-major tiles
oh[p, m*16 + f] = (dig[p, f] == m) against prebuilt iota tiles (dense
step-1 APs keep the DVE in 2x mode). The PE
accumulates the vocab count grid G[lo, b] = sum_k oh_lo(k)^T oh_b(k) over 16
chunks of 128 tokens per row (PSUM f32, exact). A vocab id is present iff
G > 0:
    setsize = sum(G_pred > 0),   overlap = sum((G_pred > 0) * (G_tgt > 0)).
Presence/reduce tails are interleaved per row into the one-hot/matmul
pipeline. Per-(partition,row) partial sums go to the host, which finishes
the tiny exact integer reductions and the final mean.

The kernel is DVE-bound (~91% busy): the is_equal one-hot build runs at the
fp16 2x-mode rate and nothing else on TRN2 can execute it (Pool rejects
TensorTensor at the ISA level; ACT has no equality op). The first row-side's
lo one-hot is emitted before the tgt digit extraction, and the last tile is
emitted in f-quarters so the final PE chain trails the DVE closely.
"""
import sys

sys.path.insert(0, "/opt/trn_rl_repo")

import numpy as np
import concourse.bass as bass
import concourse.bacc as bacc
from concourse import mybir

B = 64          # total rows
ROWS = 8        # rows per core
N_CORES = 8
L = 2048        # tokens per row
V = 32000       # vocab
P = 128
NB = 250        # b-digit range
W = P + NB      # combined one-hot width per token (378)
GSTRIDE = 256   # f32 row stride inside the PSUM grid (1KB, bank aligned)
CH = 16         # chunks of 128 tokens per row
F = mybir.dt.float32
F16 = mybir.dt.float16
I32 = mybir.dt.int32

_CACHE = {}


def _build_kernel():
    nc = bacc.Bacc()
    pred = nc.dram_tensor("pred", [ROWS, L], I32, kind="ExternalInput")
    tgt = nc.dram_tensor("tgt", [ROWS, L], I32, kind="ExternalInput")
    out = nc.dram_tensor("out", [P, 16], F, kind="ExternalOutput")

    # SBUF token layout per side: partition = (token index within row)//16,
    # free = row*16 + (token index)%16
    def tok_src(t):
        a = t[:]
        return bass.AP(a.tensor, 0, [[16, P], [L, ROWS], [1, 16]])

    from contextlib import ExitStack

    es = ExitStack()
    with es:
        sb = lambda name, shape, dt: es.enter_context(nc.sbuf_tensor(name, shape, dt))
        ps = lambda name, shape, dt: es.enter_context(nc.psum_tensor(name, shape, dt))
        sem = lambda name: es.enter_context(nc.semaphore(name))
        # both sides in one tile: cols 0..128 pred, 128..256 tgt, so digit
        # extraction runs as 4 wide ops instead of 8 narrow ones
        tok_a = sb("tok_a", [P, 2 * P], I32)
        lo_i = sb("lo_i", [P, 2 * P], I32)
        b_i = sb("b_i", [P, 2 * P], I32)
        # per side: cols off + 16r..16r+16 = row r's lo digits
        lo_a = sb("lo_a", [P, 2 * P], F16)
        bs_a = sb("bs_a", [P, 2 * P], F16)
        iota_c = sb("iota_c", [P, CH * W], F16)
        # each buffer holds a PAIR of row-side tiles (slot-major), so one-hot
        # is_equal ops cover two row-sides at once (half the per-op overhead)
        TW = CH * W
        oh0 = sb("oh0", [P, 2 * TW], F16)
        oh1 = sb("oh1", [P, 2 * TW], F16)
        oh2 = sb("oh2", [P, 2 * TW], F16)
        pres_p = sb("pres_p", [P, ROWS * NB], F16)
        pres_t = sb("pres_t", [P, ROWS * NB], F16)
        junk = sb("junk", [P, ROWS * NB], F16)
        res = sb("res", [P, 16], F)
        # one 2KB PSUM bank per tensor. Rows 1..6: bank r holds that row's
        # pred grid at [0,250) and tgt grid at [256,506). Rows 0 and 7 are
        # split across banks 0 and 7 (pred in bank0, tgt in bank7) so the
        # row-7 pred Sign can run while the PE still accumulates the row-7
        # tgt chain.
        G_r = [ps(f"G{r}", [P, 2 * GSTRIDE], F) for r in range(ROWS)]

        def g_slot(r, side):
            if r == 0:
                return (G_r[0], 0) if side == "p" else (G_r[7], 0)
            if r == ROWS - 1:
                return (G_r[0], GSTRIDE) if side == "p" else (G_r[7], GSTRIDE)
            return (G_r[r], 0 if side == "p" else GSTRIDE)
        s_load_p = sem("s_load_p")
        s_load_t = sem("s_load_t")
        s_pool = sem("s_pool")
        s_dve = sem("s_dve")
        s_pe = sem("s_pe")
        s_act = sem("s_act")
        s_out = sem("s_out")
        s_cast = sem("s_cast")
        block = es.enter_context(nc.Block())

        oh = (oh0, oh1, oh2)
        soff = {"p": 0, "t": P}
        # pair tiles T0..T5 cover rows 0-5: pred pair then tgt pair per row
        # pair j (rows 2j, 2j+1). Rows 6 and 7 stay single-tile so the final
        # chain can trail the DVE tightly. Chain order (s_pe counts): T_k ->
        # chains 2k+1, 2k+2; then 13:p6, 14:t6, 15:p7, 16:t7.
        PAIRS = [("p", 0), ("t", 0), ("p", 1), ("t", 1), ("p", 2), ("t", 2)]
        tickT = [0] * 6
        tickS = {}
        ticks = {"prod": [0] * ROWS}

        @block.gpsimd
        def _(g):
            # combined m-major iota: addr m*CH+f -> value m for m in [0,128)
            # (lo half) then [128,378) (bs half). fp16 exact for ints < 2048.
            g.iota(iota_c[:, :CH * P].rearrange("p (m f) -> p m f", f=CH),
                   pattern=[[1, P], [0, CH]], base=0, channel_multiplier=0,
                   allow_small_or_imprecise_dtypes=True).then_inc(s_pool, 1)
            HB = NB // 2
            g.iota(iota_c[:, CH * P:CH * (P + HB)].rearrange("p (m f) -> p m f", f=CH),
                   pattern=[[1, HB], [0, CH]], base=0, channel_multiplier=0,
                   allow_small_or_imprecise_dtypes=True).then_inc(s_pool, 1)
            g.iota(iota_c[:, CH * (P + HB):].rearrange("p (m f) -> p m f", f=CH),
                   pattern=[[1, NB - HB], [0, CH]], base=HB, channel_multiplier=0,
                   allow_small_or_imprecise_dtypes=True).then_inc(s_pool, 1)

        @block.vector
        def _(v):
            t = 0

            def inc(ins):
                nonlocal t
                t += 1
                return ins.then_inc(s_dve, 1)

            def digits(c0, c1, tick_key, cast_lo=False):
                # bitVec ops cannot cast on HW: compute in i32, cast via copy.
                # Only the pred lo cast (which gates the first emit) stays on
                # the DVE; the other three copies run on the ACT engine (idle
                # at kernel start), which incs s_cast per copy.
                cs = slice(c0, c1)
                inc(v.tensor_scalar(out=lo_i[:, cs], in0=tok_a[:, cs],
                                    scalar1=127, scalar2=None,
                                    op0=mybir.AluOpType.bitwise_and))
                inc(v.tensor_scalar(out=b_i[:, cs], in0=tok_a[:, cs],
                                    scalar1=7, scalar2=None,
                                    op0=mybir.AluOpType.logical_shift_right))
                ticks[tick_key] = t
                if cast_lo:
                    v.wait_ge(s_dve, t)
                    inc(v.tensor_copy(lo_a[:, cs], lo_i[:, cs]))
                    v.wait_ge(s_dve, t)

            def tail(r):
                nonlocal t
                # presence came from ACT (Sign); DVE only forms the product,
                # which ACT then reduces via Identity+accum. The LAST row's
                # tgt presence, product, and reduce all run on the DVE
                # itself (idle then) to skip ACT round-trips.
                sl = slice(NB * r, NB * (r + 1))
                if r == ROWS - 1:
                    # fused overlap indicator: min(G_t, pres_p) is 1 iff the
                    # vocab id is in both sets (G_t integer counts, pres_p in
                    # {0,1}) - one op instead of is_gt + mult
                    gt, ot_ = g_slot(r, "t")
                    v.wait_ge(s_pe, 2 * ROWS)  # final chain landed
                    v.wait_ge(s_act, 3 * ROWS - 3)  # pres_p7 (ACT tick 21)
                    inc(v.tensor_tensor(out=junk[:, sl],
                                        in0=gt[:, ot_:ot_ + NB],
                                        in1=pres_p[:, sl],
                                        op=mybir.AluOpType.min))
                    v.wait_ge(s_dve, t)
                    # free-dim reduction via tensor_scalar accum (4x mode)
                    # instead of the 1x reduce_sum
                    v.tensor_scalar(out=pres_t[:, sl], in0=junk[:, sl],
                                    scalar1=1.0, scalar2=0.0,
                                    op0=mybir.AluOpType.mult,
                                    op1=mybir.AluOpType.add,
                                    accum_out=res[:, r:r + 1])
                    t += 1
                    v.drain().then_inc(s_dve, 1)
                    ticks["final"] = t
                    return
                v.wait_ge(s_act, 2 if r == 0 else 3 * r + 1)
                inc(v.tensor_tensor(out=junk[:, sl], in0=pres_p[:, sl],
                                    in1=pres_t[:, sl],
                                    op=mybir.AluOpType.mult))
                ticks["prod"][r] = t

            def tail2(r0):
                # batched product for rows r0, r0+1 (halves the op overhead)
                nonlocal t
                sl = slice(NB * r0, NB * (r0 + 2))
                v.wait_ge(s_act, 3 * (r0 + 1) + 1)  # Sign_t(r0+1)
                inc(v.tensor_tensor(out=junk[:, sl], in0=pres_p[:, sl],
                                    in1=pres_t[:, sl],
                                    op=mybir.AluOpType.mult))
                ticks["prod"][r0] = ticks["prod"][r0 + 1] = t

            def emit_oh(buf, base, dig_src, mwidth, coloff, f0, f1):
                # oh[p, base + (coloff/CH + m)*CH + f] = (dig[p, f] == iota[m])
                nonlocal t
                o = oh[buf][:]
                v.tensor_tensor(
                    out=bass.AP(o.tensor, o.offset + base + coloff + f0,
                                [o.ap[0], [CH, mwidth], [1, f1 - f0]]),
                    in0=bass.AP(dig_src.tensor, dig_src.offset + f0,
                                [dig_src.ap[0], [0, mwidth], [1, f1 - f0]]),
                    in1=bass.AP(iota_c[:].tensor, coloff + f0,
                                [iota_c[:].ap[0], [CH, mwidth],
                                 [1, f1 - f0]]),
                    op=mybir.AluOpType.is_equal).then_inc(s_dve, 1)
                t += 1

            def emit_pair(buf, dig_t, dig_off, mwidth, coloff, sstride=16):
                # two row-side slots in one op:
                # oh[p, s*TW + coloff + m*CH + f]
                #     = (dig[p, dig_off + sstride*s + f] == m)
                nonlocal t
                o = oh[buf][:]
                da = dig_t[:]
                v.tensor_tensor(
                    out=bass.AP(o.tensor, o.offset + coloff,
                                [o.ap[0], [TW, 2], [CH, mwidth], [1, CH]]),
                    in0=bass.AP(da.tensor, da.offset + dig_off,
                                [da.ap[0], [sstride, 2], [0, mwidth], [1, CH]]),
                    in1=bass.AP(iota_c[:].tensor, coloff,
                                [iota_c[:].ap[0], [0, 2], [CH, mwidth],
                                 [1, CH]]),
                    op=mybir.AluOpType.is_equal).then_inc(s_dve, 1)
                t += 1

            HB = NB // 2
            v.wait_ge(s_load_p, 16)
            digits(0, P, "dig_p", cast_lo=True)
            # T0's lo pair only needs pred lo digits + the lo iota; emit it
            # before the tgt digit extraction to start the pipeline earlier
            v.wait_ge(s_pool, 1)  # iota lo half ready
            emit_pair(0, lo_a, 0, P, 0)
            v.wait_ge(s_load_t, 16)
            digits(P, 2 * P, "dig_t")
            # T0's b pair in m-halves; compare each as its iota lands
            v.wait_ge(s_cast, 1)  # pred b cast (on ACT) done
            v.wait_ge(s_pool, 2)
            emit_pair(0, bs_a, 0, HB, CH * P)
            v.wait_ge(s_pool, 3)
            emit_pair(0, bs_a, 0, NB - HB, CH * (P + HB))
            tickT[0] = t

            for k in range(1, 6):
                side, j = PAIRS[k]
                if k == 1:
                    v.wait_ge(s_cast, 3)  # tgt digit casts (on ACT) done
                if k >= 3:
                    v.wait_ge(s_pe, 2 * (k - 3) + 2)  # buffer k%3 consumed
                off = soff[side] + 32 * j
                emit_pair(k % 3, lo_a, off, P, 0)
                emit_pair(k % 3, bs_a, off, NB, CH * P)
                tickT[k] = t
                if k == 3:
                    tail2(0)
                elif k == 4:
                    tail2(2)

            # singles: S12=p6, S13=t6 reuse buf0 (T3, free after chain 8);
            # S14=p7, S15=t7 reuse buf1 (T4, free after chain 10)
            def dig16(side, r):
                o = soff[side] + 16 * r
                return lo_a[:, o:o + 16], bs_a[:, o:o + 16]

            v.wait_ge(s_pe, 8)
            # p6 and t6 batched as one pair (digit cols 96 and 224: uniform
            # stride 128); chains 13/14 start later but chain 15 is gated by
            # the S14 tick anyway, so the end structure is unchanged
            emit_pair(0, lo_a, soff["p"] + 96, P, 0, sstride=P)
            emit_pair(0, bs_a, soff["p"] + 96, NB, CH * P, sstride=P)
            tickS[12] = tickS[13] = t
            tail2(4)
            v.wait_ge(s_pe, 10)
            lo_s, bs_s = dig16("p", 7)
            emit_oh(1, 0, lo_s, P, 0, 0, CH)
            emit_oh(1, 0, bs_s, NB, CH * P, 0, CH)
            tickS[14] = t
            # last row-side in f-quarters so the final PE chain trails the
            # DVE closely instead of waiting for the full tile; row 6's
            # product goes AFTER the emits, into the wait-for-final-chain gap
            lo_s, bs_s = dig16("t", 7)
            ticks["oh_q"] = {}
            for f0, f1 in ((0, 8), (8, 12), (12, 14), (14, 16)):
                emit_oh(1, TW, lo_s, P, 0, f0, f1)
                emit_oh(1, TW, bs_s, NB, CH * P, f0, f1)
                ticks["oh_q"][f1] = t
            tail(6)

            tail(ROWS - 1)

        @block.scalar
        def _(sc):

            # tgt digit casts offloaded from the DVE (ACT is idle here);
            # signalled via s_pool so the s_act Sign/Id tick numbering is
            # untouched
            sc.wait_ge(s_dve, ticks["dig_p"])
            sc.activation(bs_a[:, :P], b_i[:, :P],
                          mybir.ActivationFunctionType.Copy,
                          ).then_inc(s_cast, 1)
            sc.wait_ge(s_dve, ticks["dig_t"])
            sc.activation(lo_a[:, P:], lo_i[:, P:],
                          mybir.ActivationFunctionType.Copy,
                          ).then_inc(s_cast, 1)
            sc.activation(bs_a[:, P:], b_i[:, P:],
                          mybir.ActivationFunctionType.Copy,
                          ).then_inc(s_cast, 1)
            # chain numbering under the pair schedule: rows r<6 have
            # c_p = 4(r//2)+1+r%2, c_t = c_p+2; then 13:p6, 14:t6, 15:p7,
            # 16:t7. A bank may be read only when ALL its writer chains are
            # done (rows 1..6 share a bank between their p and t grids; rows
            # 0 and 7 use the split banks, readable after their own chain).
            W_p = {0: 1, 1: 4, 2: 7, 3: 8, 4: 11, 5: 12, 6: 14, 7: 15}
            W_t = {0: 3, 1: 4, 2: 7, 3: 8, 4: 11, 5: 12, 6: 14}
            for r in range(ROWS):
                sl = slice(NB * r, NB * (r + 1))
                gp, op_ = g_slot(r, "p")
                gt, ot_ = g_slot(r, "t")
                sc.wait_ge(s_pe, W_p[r])
                sc.activation(pres_p[:, sl], gp[:, op_:op_ + NB],
                              mybir.ActivationFunctionType.Sign,
                              accum_out=res[:, ROWS + r:ROWS + r + 1],
                              ).then_inc(s_act, 1)
                if r < ROWS - 1:
                    sc.wait_ge(s_pe, W_t[r])
                    sc.activation(pres_t[:, sl], gt[:, ot_:ot_ + NB],
                                  mybir.ActivationFunctionType.Sign,
                                  ).then_inc(s_act, 1)
                if r >= 1:
                    sc.wait_ge(s_dve, ticks["prod"][r - 1])
                    sc.activation(junk[:, NB * (r - 1):NB * r],
                                  junk[:, NB * (r - 1):NB * r],
                                  mybir.ActivationFunctionType.Identity,
                                  accum_out=res[:, r - 1:r],
                                  ).then_inc(s_act, 1)

        @block.tensor
        def _(te):
            def chain(bufi, base, gten, goff, last=False):
                a = oh[bufi][:]
                nxt = {8: 12, 12: 14, 14: 16}
                for f in range(CH):
                    if last and f in nxt:
                        te.wait_ge(s_dve, ticks["oh_q"][nxt[f]])
                    ins = te.matmul(
                        out=gten[:, goff:goff + NB],
                        lhsT=bass.AP(a.tensor, a.offset + base + f,
                                     [a.ap[0], [CH, P]]),
                        rhs=bass.AP(a.tensor, a.offset + base + CH * P + f,
                                    [a.ap[0], [CH, NB]]),
                        start=(f == 0),
                        stop=(f == CH - 1),
                    )
                    if f == CH - 1:
                        ins.then_inc(s_pe, 1)

            for k in range(6):
                side, j = PAIRS[k]
                te.wait_ge(s_dve, tickT[k])
                for s in range(2):
                    g, go = g_slot(2 * j + s, side)
                    chain(k % 3, s * TW, g, go)
            te.wait_ge(s_dve, tickS[12])
            g, go = g_slot(6, "p")
            chain(0, 0, g, go)
            te.wait_ge(s_dve, tickS[13])
            g, go = g_slot(6, "t")
            chain(0, TW, g, go)
            te.wait_ge(s_dve, tickS[14])
            g, go = g_slot(7, "p")
            chain(1, 0, g, go)
            te.wait_ge(s_dve, ticks["oh_q"][8])
            g, go = g_slot(7, "t")
            chain(1, TW, g, go, last=True)

        @block.sync
        def _(sy):
            sy.dma_start(out=tok_a[:, :P].rearrange("p (a g) -> p a g", g=16),
                         in_=tok_src(pred)).then_inc(s_load_p, 16)
            # tgt load issued second on the same engine: pred's DMA wins the
            # shared HWDGE deterministically (tgt digits aren't read until
            # after the first lo emit, ~5us in)
            sy.dma_start(out=tok_a[:, P:].rearrange("p (a g) -> p a g", g=16),
                         in_=tok_src(tgt)).then_inc(s_load_t, 16)
            # rows 0..5 fully reduced once red5 fired (ACT tick 3*6+2=20)
            sy.wait_ge(s_act, 20)
            ra = res[:]
            sy.dma_start(
                out=bass.AP(out[:].tensor, 0, [[16, P], [ROWS, 2], [1, 6]]),
                in_=bass.AP(ra.tensor, 0, [ra.ap[0], [ROWS, 2], [1, 6]]),
            ).then_inc(s_out, 16)
            sy.wait_ge(s_act, 3 * ROWS - 2)
            sy.wait_ge(s_dve, ticks["final"])
            sy.dma_start(
                out=bass.AP(out[:].tensor, 6, [[16, P], [ROWS, 2], [1, 2]]),
                in_=bass.AP(ra.tensor, 6, [ra.ap[0], [ROWS, 2], [1, 2]]),
            ).then_inc(s_out, 16)
            sy.wait_ge(s_out, 32)

    nc.compile()
    return nc


def run(pred_tokens, tgt_tokens, trace=False):
    """Returns (loss, exec_time_ns_or_None)."""
    from concourse.bass_utils import run_bass_kernel_spmd

    if "nc" not in _CACHE:
        _CACHE["nc"] = _build_kernel()
    nc = _CACHE["nc"]

    pred_tokens = np.ascontiguousarray(np.asarray(pred_tokens, dtype=np.int32))
    tgt_tokens = np.ascontiguousarray(np.asarray(tgt_tokens, dtype=np.int32))
    assert pred_tokens.shape == (B, L) and tgt_tokens.shape == (B, L)

    in_maps = [
        {
            "pred": pred_tokens[c * ROWS:(c + 1) * ROWS],
            "tgt": tgt_tokens[c * ROWS:(c + 1) * ROWS],
        }
        for c in range(N_CORES)
    ]
    try:
        kres = run_bass_kernel_spmd(nc, in_maps, list(range(N_CORES)),
                                    trace=trace)
    except ModuleNotFoundError:
        # NTFF profiling hook unavailable in this axon client
        kres = run_bass_kernel_spmd(nc, in_maps, list(range(N_CORES)))

    ov = np.empty(B, dtype=np.float32)
    ss = np.empty(B, dtype=np.float32)
    for c, r in enumerate(kres.results):
        o = r["out"]  # [128, 16] f32: cols 0..7 overlap partials, 8..15 setsize
        ov[c * ROWS:(c + 1) * ROWS] = o[:, :ROWS].sum(axis=0, dtype=np.float64)
        ss[c * ROWS:(c + 1) * ROWS] = o[:, ROWS:].sum(axis=0, dtype=np.float64)

    precision = np.where(ss > 0, ov / np.maximum(ss, np.float32(1.0)),
                         np.float32(0.0)).astype(np.float32)
    loss = np.float32(1.0) - np.float32(precision.mean(dtype=np.float64))
    return loss, kres.exec_time_ns


def kernel(pred_tokens, target_tokens):
    loss, _ = run(pred_tokens, target_tokens)
    return loss


if __name__ == "__main__":
    rng = np.random.default_rng(0)
    p = rng.integers(0, V, (B, L), dtype=np.int32)
    t = rng.integers(0, V, (B, L), dtype=np.int32)
    print(kernel(p, t))

